# revision 47
# baseline (speedup 1.0000x reference)
"""Trainium2 Bass kernel for nn_AggregateGCN (3-layer GCN, batched graph,
agent-node readout).

Math (reference): deg-normalized GraphConv x2 on top of a linear+relu input
projection, then a final projection of the 64 agent rows (nodes 0, N, 2N, ...).
Only the 64 agent rows of the last conv are read, so the exact computation
is the backward dependency cone:
  layer2 needs edges into the 64 agents (~2k edges -> ~2k distinct sources S1)
  layer1 needs edges into S1 (~64k edges), with per-edge h0 = relu(x@w_lin+b)

Sharding: agents are LPT-assigned to cores (8 each, balancing cone edge
counts) with each core's full cone replicated -> zero cross-device traffic;
the host scatters the per-core [8, 64] outputs back to global row order.

All biases are zero for this problem, which makes the whole pipeline
positively homogeneous: relu(s*x @ W) = s*relu(x @ W) for s>0. The host
therefore folds the per-edge out-degree norm into x itself and ships
  xeT = (x[src] * out_norm[src])^T  in bf16
so the device does NO degree math at all. All matmuls run in bf16
(fp32 matmul is 4 cycles/row on TRN2; bf16 is 1), accumulating in fp32 PSUM.

On device per core (per 128-edge chunk):
  - h0 = xe_chunk.T @ wlin           (PE, bf16, N=256)
  - hs0 = relu(h0)                   (ACT or DVE, alternating; bf16 out)
  - S[e,d] = (iota[d]==dstl[e])      (DVE tensor_scalar, bf16, exact 0/1)
  - aggT[f,d] += hs0_fhalf.T @ S     (PE, accumulated in PSUM per half)
then per half: h1 = aggT.T @ wc0, hs1 = relu(io1 * h1) with io1 =
in_norm*out_norm per slot (host-computed; legal because relu(c*z)=c*relu(z)
for c>0), and a small feature-major stage B for the 8 agent rows.
"""
import sys

sys.path.insert(0, "/opt/trn_rl_repo")

import numpy as np
import concourse.bass as bass
import concourse.bacc as bacc
import concourse.mybir as mybir
import concourse.tile as tile

F32 = mybir.dt.float32
BF16 = mybir.dt.bfloat16
AF = mybir.ActivationFunctionType
ALU = mybir.AluOpType

# problem constants (fixed by the spec)
B = 64          # graphs
NPG = 2048      # nodes per graph
TOTAL = B * NPG
IN_DIM = 128
HID = 256
EMB = 64
NCORES = 8
AGENTS_PER_CORE = B // NCORES      # 8
M1 = 384                           # padded S1 slots per core (3 halves of 128)
NHALF = M1 // 128                  # 3
P = 128


def _meta16_cols(nchunk):
    """Column offsets inside the packed bf16 meta tensor [128, MW]."""
    off = {}
    o = 0
    for name, w in [("iota", P),
                    ("a2t", NHALF * AGENTS_PER_CORE), ("wlin", HID),
                    ("wblk", 2 * (HID + HID + EMB))]:
        off[name] = (o, o + w)
        o += w
    return off, o


def build_program(nch_per_half: int, repeat: int = 1,
                  zero_bias: bool = True, ablate: str = "") -> bass.Bass:
    """One SPMD program; per-core data differs via in_maps. repeat>1 re-runs
    the whole compute in a hardware For_i loop (for slope-based HW timing).
    ablate: diagnostic-only comma list ('noh0','nospmm','nosbuild','noevict')
    that skips parts of the pipeline (output becomes garbage)."""
    assert zero_bias, "bf16 fast path requires zero biases"
    nchunk = NHALF * nch_per_half
    ne = nchunk * P
    moff, mw16 = _meta16_cols(nchunk)

    nc = bacc.Bacc(
        "TRN2", target_bir_lowering=False, debug=False, num_devices=NCORES
    )
    # per-core inputs
    xeT = nc.declare_dram_parameter("xeT", [P, ne], BF16, isOutput=False)
    meta16 = nc.declare_dram_parameter("meta16", [P, mw16], BF16, isOutput=False)
    meta32 = nc.declare_dram_parameter("meta32", [P, NHALF + 1 + nchunk], F32,
                                       isOutput=False)
    out = nc.declare_dram_parameter("out", [AGENTS_PER_CORE, EMB], F32,
                                    isOutput=True)

    with tile.TileContext(nc) as tc:
        with (
            tc.tile_pool(name="const", bufs=1) as cp,
            tc.tile_pool(name="io", bufs=2) as iop,
            tc.tile_pool(name="hs0p", bufs=5) as hs0p,
            tc.tile_pool(name="selp", bufs=8) as selp,
            tc.tile_pool(name="stage", bufs=1) as stp,
            tc.tile_pool(name="h0ps", bufs=3, space="PSUM") as h0psp,
            tc.tile_pool(name="aggtps", bufs=2, space="PSUM") as aggtpsp,
            tc.tile_pool(name="mlpps", bufs=1, space="PSUM") as mlppsp,
        ):
            def _body():
                emit_compute(
                    nc, cp, iop, hs0p, selp, stp, h0psp, aggtpsp, mlppsp,
                    nch_per_half, nchunk, ne, moff, mw16,
                    xeT, meta16, meta32, out, ablate,
                )

            if repeat == 1:
                _body()
            else:
                with tc.For_i(0, repeat, 1):
                    _body()
    nc.compile()
    return nc


def emit_compute(nc, cp, iop, hs0p, selp, stp, h0psp, aggtpsp, mlppsp,
                 nch_per_half, nchunk, ne, moff, mw16,
                 xeT, meta16, meta32, out, ablate=""):
    abl = set(a for a in ablate.split(",") if a)
    AG = AGENTS_PER_CORE
    if "empty" in abl:     # diagnostic: measure bare For_i loop overhead
        out_e = stp.tile([AG, EMB], F32, tag="outt")
        nc.vector.memset(out_e[:], 0.0)
        nc.sync.dma_start(out=out[:], in_=out_e[:])
        return
    KW = HID + HID + EMB               # 576: packed width per k-half of wblk

    # ---- all small constants ride ahead of the bulk xeT ----
    # double-buffered (iop pool) so iteration i+1's DMAs overlap iteration
    # i's compute in the For_i timing loop instead of serializing behind
    # its last reader
    meta16_t = iop.tile([P, mw16], BF16, tag="meta16")
    nc.sync.dma_start(out=meta16_t[:], in_=meta16[:])
    meta32_t = iop.tile([P, NHALF + 1 + nchunk], F32, tag="meta32")
    nc.sync.dma_start(out=meta32_t[:], in_=meta32[:])

    def mslice(name):
        lo, hi = moff[name]
        return meta16_t[:, lo:hi]

    dstl_t = meta32_t[:, NHALF + 1:]
    iota_t = mslice("iota")
    wblk_t = mslice("wblk").rearrange("p (c n) -> p c n", n=KW)
    wlin_t = mslice("wlin")
    a2t_t = mslice("a2t").rearrange("p (c n) -> p c n", n=AG)
    io1_t = meta32_t[:, 0:NHALF]       # per-slot in_norm*out_norm, per half
    in2_t = meta32_t[:AG, NHALF:NHALF + 1]   # per-agent in_norm

    # GPSIMD S-builds for every 4th chunk, pre-issued at iteration start so
    # the slow GPSIMD queue has a full iteration of lead time over the PE
    gchunks = [c for c in range(nchunk) if c % 4 == 3]
    gidx = {c: i for i, c in enumerate(gchunks)}
    gss_t = None
    if gchunks and "nosbuild" not in abl and "nospmm" not in abl:
        gss_t = iop.tile([P, len(gchunks) * P], BF16, tag="gss")
        for c, i in gidx.items():
            nc.gpsimd.tensor_scalar(
                out=gss_t[:, i * P:(i + 1) * P], in0=iota_t,
                scalar1=dstl_t[:, c:c + 1], scalar2=None,
                op0=ALU.is_equal,
            )

    # PE warm-up: keep the HAM activity window busy while DMAs stream in
    wu_t = cp.tile([P, P], BF16, tag="wu")
    nc.vector.memset(wu_t[:], 0.25)
    warm_ps = mlppsp.tile([P, HID], F32, tag="mlp", name="warm")
    for _w in range(3):
        nc.tensor.matmul(
            out=warm_ps[:, :P], lhsT=wu_t[:], rhs=wu_t[:],
            start=True, stop=True,
        )

    # ---- bulk x_e^T load, sliced so compute starts after slice 0 ----
    xeT_t = iop.tile([P, ne], BF16, tag="xeT")
    n_sl = 2
    sl = -(-nchunk // n_sl) * P   # slice width in columns, chunk-aligned
    for _s in range(n_sl):
        lo, hi = _s * sl, min((_s + 1) * sl, ne)
        if lo >= hi:
            break
        nc.sync.dma_start(out=xeT_t[:, lo:hi], in_=xeT[:, lo:hi])

    # ---- stage A: per half, accumulate aggT then h1 ----
    # h1 for half h-1 is emitted after half h's first pairs so the PE FIFO
    # never stalls on the DVE aggT evictions at half boundaries.
    hs1_t = stp.tile([P, NHALF, HID], BF16, tag="hs1")  # node-major
    assert nch_per_half % 2 == 0

    def emit_h1(aggT_prev, hprev):
        h1_ps = mlppsp.tile([P, HID], F32, tag="mlp", name=f"h1_{hprev}")
        aggT_sb = selp.tile([P, 2 * P], BF16, tag="daT")
        nc.vector.tensor_copy(out=aggT_sb[:], in_=aggT_prev[:])
        for k in range(HID // P):
            nc.tensor.matmul(
                out=h1_ps[:], lhsT=aggT_sb[:, k * P:(k + 1) * P],
                rhs=wblk_t[:, k, 0:HID],
                start=(k == 0), stop=(k == HID // P - 1),
            )
        nc.scalar.activation(
            hs1_t[:, hprev, :], h1_ps[:], AF.Relu, scale=io1_t[:, hprev:hprev + 1]
        )

    h0_pre = []
    if "noh0" in abl:     # 3 pre-written PSUM tiles the evictions can read
        for _i in range(3):
            t = h0psp.tile([P, 2 * HID], F32, tag="h0", name=f"h0pre{_i}")
            for u in range(2):
                nc.tensor.matmul(
                    out=t[:, u * HID:(u + 1) * HID],
                    lhsT=xeT_t[:, u * P:(u + 1) * P], rhs=wlin_t,
                    start=True, stop=True)
            h0_pre.append(t)

    prev = None
    for h in range(NHALF):
        # both f-halves of aggT side by side in one PSUM bank
        aggT_ps = aggtpsp.tile([P, 2 * P], F32, tag="aggT", name=f"aggT_{h}")
        if "nospmm" in abl:       # keep h1's producers defined
            for fh in range(2):
                nc.tensor.matmul(out=aggT_ps[:, fh * P:(fh + 1) * P],
                                 lhsT=iota_t, rhs=iota_t,
                                 start=True, stop=True)
        for jp in range(nch_per_half // 2):
            c0 = h * nch_per_half + 2 * jp
            if "noh0" in abl:
                h0_ps = h0_pre[jp % 3]
            else:
                h0_ps = h0psp.tile([P, 2 * HID], F32, tag="h0")
                for u in range(2):
                    nc.tensor.matmul(
                        out=h0_ps[:, u * HID:(u + 1) * HID],
                        lhsT=xeT_t[:, (c0 + u) * P:(c0 + u + 1) * P],
                        rhs=wlin_t,
                        start=True, stop=True,
                    )
            hs0_t = None
            if "noevict" not in abl:
                hs0_t = hs0p.tile([P, 2 * HID], BF16, tag="hs0")
                if jp % 2 == 1:
                    # every third pair's relu eviction runs on DVE to keep
                    # the ACT queue from becoming the bottleneck
                    nc.vector.tensor_scalar(
                        out=hs0_t[:], in0=h0_ps[:], scalar1=0.0, scalar2=None,
                        op0=ALU.max,
                    )
                else:
                    nc.scalar.activation(hs0_t[:], h0_ps[:], AF.Relu)
            if "nospmm" not in abl:
                for u in range(2):
                    c = c0 + u
                    j = 2 * jp + u
                    # S[e,d] = (iota[d] == dstl[e]): one DVE op, bf16 exact
                    if "nosbuild" in abl:
                        ss_t = iota_t
                    elif gss_t is not None and c in gidx:
                        gi = gidx[c]
                        ss_t = gss_t[:, gi * P:(gi + 1) * P]
                    else:
                        ss_t = selp.tile([P, P], BF16, tag="ssel")
                        nc.vector.tensor_scalar(
                            out=ss_t[:], in0=iota_t,
                            scalar1=dstl_t[:, c:c + 1], scalar2=None,
                            op0=ALU.is_equal,
                        )
                    for fh in range(2):
                        lh = (hs0_t[:, u * HID + fh * P:u * HID + (fh + 1) * P]
                              if hs0_t is not None else iota_t[:])
                        # one accumulation group spans both column halves:
                        # start clears the whole bank; later MMs overwrite
                        # untouched elements via per-element has_written
                        nc.tensor.matmul(
                            out=aggT_ps[:, fh * P:(fh + 1) * P],
                            lhsT=lh,
                            rhs=ss_t[:] if ss_t is not iota_t else iota_t[:],
                            start=(j == 0 and fh == 0),
                            stop=(j == nch_per_half - 1 and fh == 1),
                        )
            if jp == 1 and prev is not None:
                emit_h1(*prev)
                prev = None
        prev = (aggT_ps, h)
    emit_h1(*prev)

    # ---- stage B: layer 2 on the 8 agent rows, feature-major ----
    out_ps = mlppsp.tile([AG, EMB], F32, tag="mlp", name="outps")
    h2rT_t = [None, None]
    for oh in range(2):
        a2T_ps = aggtpsp.tile([P, AG], F32, tag="aggT",
                              name=f"a2T{oh}")
        for h in range(NHALF):
            nc.tensor.matmul(
                out=a2T_ps[:],
                lhsT=hs1_t[:, h, oh * P:(oh + 1) * P],
                rhs=a2t_t[:, h, :],
                start=(h == 0), stop=(h == NHALF - 1),
            )
        a2T_sb = selp.tile([P, AG], BF16, tag="da2T", name=f"a2Tsb{oh}")
        nc.vector.tensor_copy(out=a2T_sb[:], in_=a2T_ps[:])
        h2rT_t[oh] = a2T_sb
    z2T_sb = [None, None]
    for oh in range(2):
        z2_ps = aggtpsp.tile([P, AG], F32, tag="aggT",
                             name=f"z2T{oh}")
        for kc in range(2):
            nc.tensor.matmul(
                out=z2_ps[:],
                lhsT=wblk_t[:, kc, HID + oh * P:HID + (oh + 1) * P],
                rhs=h2rT_t[kc][:],
                start=(kc == 0), stop=(kc == 1),
            )
        zr_t = hs0p.tile([P, AG], BF16, tag="hb", name=f"z2r{oh}")
        nc.scalar.activation(zr_t[:], z2_ps[:], AF.Relu)
        z2T_sb[oh] = zr_t
    for oh in range(2):
        nc.tensor.matmul(
            out=out_ps[:], lhsT=z2T_sb[oh][:],
            rhs=wblk_t[:, oh, 2 * HID:2 * HID + EMB],
            start=(oh == 0), stop=(oh == 1),
        )
    out_t = stp.tile([AG, EMB], F32, tag="outt")
    nc.scalar.activation(out_t[:], out_ps[:], AF.Copy, scale=in2_t[:, 0:1])
    nc.sync.dma_start(out=out[:], in_=out_t[:])


def prepare_inputs(x, src, dst):
    """Host-side integer index preprocessing + sharding. Agents are
    LPT-assigned to cores (8 each) to balance cone edge counts, and S1
    nodes are LPT-assigned to the 3 dst halves to balance chunk counts."""
    deg_out = np.bincount(src, minlength=TOTAL).astype(np.float32)
    deg_in = np.bincount(dst, minlength=TOTAL).astype(np.float32)

    g = dst // NPG                     # graph id of each edge's dst
    is_agent = (dst % NPG) == 0
    g2 = g[is_agent]
    s2_all = src[is_agent]

    # per-agent cone load = sum of in-degrees over its distinct sources
    loads = np.zeros(B, np.int64)
    for a in range(B):
        loads[a] = deg_in[np.unique(s2_all[g2 == a])].sum()
    bins = [[] for _ in range(NCORES)]
    bl = np.zeros(NCORES, np.int64)
    for a in np.argsort(-loads):
        cands = [i for i in range(NCORES) if len(bins[i]) < AGENTS_PER_CORE]
        i = min(cands, key=lambda i: bl[i])
        bins[i].append(int(a))
        bl[i] += loads[a]

    cores = []
    agent_rows = []                     # global output row per concat position
    nch_needed = 1
    for c in range(NCORES):
        agents_g = bins[c]              # graph ids owned by this core
        agent_rows.extend(agents_g)
        # --- layer-2 edge bucket: dst is an agent owned by this core ---
        am = np.zeros(B, bool)
        am[agents_g] = True
        m2 = is_agent & am[g]
        e2_src = src[m2]
        gl = np.full(B, -1, np.int64)
        gl[agents_g] = np.arange(AGENTS_PER_CORE)
        e2_ag = gl[g[m2]]
        s1 = np.unique(e2_src)
        m1c = s1.size
        assert m1c <= NHALF * 127, f"S1 overflow: {m1c}"
        # slot: LPT nodes into halves by in-degree (127 usable slots each,
        # slot 127 of each half is the pad/trash slot)
        hload = np.zeros(NHALF, np.int64)
        hfill = np.zeros(NHALF, np.int64)
        slot = np.empty(m1c, np.int64)
        d1 = deg_in[s1].astype(np.int64)
        for i in np.argsort(-d1):
            cands = [hh for hh in range(NHALF) if hfill[hh] < P - 1]
            hh = min(cands, key=lambda hh: hload[hh])
            slot[i] = hh * P + hfill[hh]
            hfill[hh] += 1
            hload[hh] += d1[i]
        # lookup: global node id -> slot
        loc = np.full(TOTAL, -1, dtype=np.int64)
        loc[s1] = slot
        a2t = np.zeros((M1, AGENTS_PER_CORE), dtype=np.float32)
        np.add.at(a2t, (loc[e2_src], e2_ag), 1.0)

        indeg1 = np.zeros(M1, np.float32)
        outdeg1 = np.zeros(M1, np.float32)
        indeg1[loc[s1]] = deg_in[s1]
        outdeg1[loc[s1]] = deg_out[s1]
        agents = np.asarray(agents_g, np.int64) * NPG
        indeg2 = deg_in[agents].reshape(AGENTS_PER_CORE, 1)

        # --- layer-1 edge bucket: dst in S1 ---
        dl = loc[dst]
        sel = dl >= 0
        e1_src = src[sel]
        e1_slot = dl[sel]
        halves = []
        for h in range(NHALF):
            hm = (e1_slot // P) == h
            halves.append((e1_src[hm], e1_slot[hm] - h * P))
            nch_needed = max(nch_needed, -(-halves[h][0].size // P))
        cores.append(dict(a2t=a2t, indeg1=indeg1, outdeg1=outdeg1,
                          indeg2=indeg2, halves=halves))
    return cores, deg_out, nch_needed, np.asarray(agent_rows, np.int64)


def pack_core(core, x, out_norm, nch_per_half, w_lin, wblk, np16):
    nchunk = NHALF * nch_per_half
    ne = nchunk * P
    moff, mw16 = _meta16_cols(nchunk)
    xe = np.zeros((ne, IN_DIM), dtype=np.float32)
    dstl_e = np.full(ne, P - 1, dtype=np.float32)  # pads -> trash slot 127
    for h, (hsrc, hslot) in enumerate(core["halves"]):
        base = h * nch_per_half * P
        k = hsrc.size
        # out-degree norm folded into x (exact: relu(s*z) = s*relu(z), s>0)
        xe[base:base + k] = x[hsrc] * out_norm[hsrc][:, None]
        dstl_e[base:base + k] = hslot
    meta16 = np.zeros((P, mw16), dtype=np.float32)

    def put(name, arr):
        lo, hi = moff[name]
        meta16[:arr.shape[0], lo:hi] = arr

    put("iota", np.broadcast_to(np.arange(P, dtype=np.float32), (P, P)))
    put("a2t", core["a2t"].reshape(NHALF, P, AGENTS_PER_CORE)
        .transpose(1, 0, 2).reshape(P, NHALF * AGENTS_PER_CORE))
    put("wlin", np.asarray(w_lin, np.float32))
    put("wblk", wblk)

    # meta32: io1 (per-slot in*out norm, [128, 3]) + in2 (per-agent, rows 0-7)
    i1 = 1.0 / np.sqrt(np.maximum(core["indeg1"], 1.0))
    o1 = 1.0 / np.sqrt(np.maximum(core["outdeg1"], 1.0))
    io1 = (i1 * o1).reshape(NHALF, P).T
    meta32 = np.zeros((P, NHALF + 1 + nchunk), dtype=np.float32)
    meta32[:, 0:NHALF] = io1
    meta32[:AGENTS_PER_CORE, NHALF:NHALF + 1] = \
        1.0 / np.sqrt(np.maximum(core["indeg2"], 1.0))
    meta32[:, NHALF + 1:] = dstl_e.reshape(nchunk, P).T
    return dict(xeT=np.ascontiguousarray(xe.T).astype(np16),
                meta16=meta16.astype(np16), meta32=meta32)


def pack_weights(w_c0, w_c1, w_emb):
    """[128, 2, wc0(256)|wc1(256)|wemb(64)] k-major, fp32, flattened."""
    KW = HID + HID + EMB
    wb = np.zeros((P, 2, KW), np.float32)
    for k in range(2):
        wb[:, k, 0:HID] = np.asarray(w_c0, np.float32)[k * P:(k + 1) * P]
        wb[:, k, HID:2 * HID] = np.asarray(w_c1, np.float32)[k * P:(k + 1) * P]
        wb[:, k, 2 * HID:] = np.asarray(w_emb, np.float32)[k * P:(k + 1) * P]
    return wb.reshape(P, 2 * KW)


def assemble_out(core_outs, agent_rows):
    """Scatter per-core [8, EMB] outputs back to global agent row order."""
    full = np.empty((B, EMB), np.float32)
    full[agent_rows] = np.concatenate(core_outs, axis=0)
    return full


def make_in_maps(x, src, dst, w_lin, b_lin, w_c0, b_c0, w_c1, b_c1,
                 w_emb, b_emb):
    """Host preprocessing -> (in_maps, nch_per_half, zero_bias, agent_rows)."""
    np16 = mybir.dt.np(BF16)
    x = np.asarray(x, dtype=np.float32)
    src = np.asarray(src).astype(np.int64)
    dst = np.asarray(dst).astype(np.int64)
    cores, deg_out, nch_per_half, agent_rows = prepare_inputs(x, src, dst)
    nch_per_half += nch_per_half % 2   # paired-chunk path needs even count
    out_norm = 1.0 / np.sqrt(np.maximum(deg_out, 1.0))
    wblk = pack_weights(w_c0, w_c1, w_emb)
    in_maps = []
    for c in range(NCORES):
        m = pack_core(cores[c], x, out_norm, nch_per_half, w_lin, wblk, np16)
        in_maps.append(m)
    return in_maps, nch_per_half, True, agent_rows


def _host_fallback(x, src, dst, w_lin, b_lin, w_c0, b_c0, w_c1, b_c1,
                   w_emb, b_emb):
    """Numpy fallback for nonzero biases (never hit for this problem)."""
    x = np.asarray(x, np.float32)
    src = np.asarray(src).astype(np.int64)
    dst = np.asarray(dst).astype(np.int64)
    deg_out = np.bincount(src, minlength=TOTAL).astype(np.float32)
    deg_in = np.bincount(dst, minlength=TOTAL).astype(np.float32)
    on = np.maximum(deg_out, 1.0) ** -0.5
    inn = np.maximum(deg_in, 1.0) ** -0.5

    def conv(h, W, bb):
        hs = h * on[:, None]
        agg = np.zeros_like(h)
        for f in range(h.shape[1]):
            agg[:, f] = np.bincount(dst, weights=hs[src, f].astype(np.float64),
                                    minlength=TOTAL)
        return (agg * inn[:, None]) @ W + bb

    h = np.maximum(x @ w_lin + b_lin, 0.0)
    h = np.maximum(conv(h, w_c0, b_c0), 0.0)
    h = np.maximum(conv(h, w_c1, b_c1), 0.0)
    return (h[::NPG] @ w_emb + b_emb).astype(np.float32)


def kernel(x, src, dst, num_nodes, nodes_per_graph,
           w_lin, b_lin, w_c0, b_c0, w_c1, b_c1, w_emb, b_emb,
           _debug=None) -> np.ndarray:
    from concourse.bass_utils import run_bass_kernel_spmd

    assert int(num_nodes) == TOTAL and int(nodes_per_graph) == NPG
    if (np.any(np.asarray(b_lin)) or np.any(np.asarray(b_c0))
            or np.any(np.asarray(b_c1)) or np.any(np.asarray(b_emb))):
        return _host_fallback(x, src, dst, w_lin, b_lin, w_c0, b_c0,
                              w_c1, b_c1, w_emb, b_emb)
    in_maps, nch_per_half, zero_bias, agent_rows = make_in_maps(
        x, src, dst, w_lin, b_lin, w_c0, b_c0, w_c1, b_c1, w_emb, b_emb)

    nc = build_program(nch_per_half, zero_bias=zero_bias)
    core_ids = list(range(NCORES))
    if _debug is not None:
        _debug["nc"] = nc
        _debug["in_maps"] = in_maps
        _debug["nch_per_half"] = nch_per_half
    res = run_bass_kernel_spmd(nc, in_maps, core_ids)
    return assemble_out([res.results[c]["out"] for c in range(NCORES)],
                        agent_rows)


# revision 50
# speedup vs baseline: 2.0218x; 2.0218x over previous
"""Trainium2 Bass kernel for nn_AggregateGCN (3-layer GCN, batched graph,
agent-node readout).

Math (reference): deg-normalized GraphConv x2 on top of a linear+relu input
projection, then a final projection of the 64 agent rows (nodes 0, N, 2N, ...).
Only the 64 agent rows of the last conv are read, so the exact computation
is the backward dependency cone:
  layer2 needs edges into the 64 agents (~2k edges -> ~2k distinct sources S1)
  layer1 needs edges into S1 (~64k edges), with per-edge h0 = relu(x@w_lin+b)

Sharding: agents are LPT-assigned to cores (8 each, balancing cone edge
counts) with each core's full cone replicated -> zero cross-device traffic;
the host scatters the per-core [8, 64] outputs back to global row order.

All biases are zero for this problem, which makes the whole pipeline
positively homogeneous: relu(s*x @ W) = s*relu(x @ W) for s>0. The host
therefore folds the per-edge out-degree norm into x itself and ships
  xeT = (x[src] * out_norm[src])^T  in bf16
so the device does NO degree math at all. All matmuls run in bf16
(fp32 matmul is 4 cycles/row on TRN2; bf16 is 1), accumulating in fp32 PSUM.

On device per core (per 128-edge chunk):
  - h0 = xe_chunk.T @ wlin           (PE, bf16, N=256)
  - hs0 = relu(h0)                   (ACT or DVE, alternating; bf16 out)
  - S[e,d] = (iota[d]==dstl[e])      (DVE tensor_scalar, bf16, exact 0/1)
  - aggT[f,d] += hs0_fhalf.T @ S     (PE, accumulated in PSUM per half)
then per half: h1 = aggT.T @ wc0, hs1 = relu(io1 * h1) with io1 =
in_norm*out_norm per slot (host-computed; legal because relu(c*z)=c*relu(z)
for c>0), and a small feature-major stage B for the 8 agent rows.
"""
import sys

sys.path.insert(0, "/opt/trn_rl_repo")

import numpy as np
import concourse.bass as bass
import concourse.bacc as bacc
import concourse.mybir as mybir
import concourse.tile as tile

F32 = mybir.dt.float32
BF16 = mybir.dt.bfloat16
AF = mybir.ActivationFunctionType
ALU = mybir.AluOpType

# problem constants (fixed by the spec)
B = 64          # graphs
NPG = 2048      # nodes per graph
TOTAL = B * NPG
IN_DIM = 128
HID = 256
EMB = 64
NCORES = 8
AGENTS_PER_CORE = B // NCORES      # 8
M1 = 384                           # padded S1 slots per core (3 halves of 128)
NHALF = M1 // 128                  # 3
P = 128


def _meta16_cols(nchunk):
    """Column offsets inside the packed bf16 meta tensor [128, MW]."""
    off = {}
    o = 0
    for name, w in [("iota", P),
                    ("a2t", NHALF * AGENTS_PER_CORE), ("wlin", HID),
                    ("wblk", 2 * (HID + HID + EMB))]:
        off[name] = (o, o + w)
        o += w
    return off, o


def build_program(nch_per_half: int, repeat: int = 1,
                  zero_bias: bool = True, ablate: str = "") -> bass.Bass:
    """One SPMD program; per-core data differs via in_maps. repeat>1 re-runs
    the whole compute in a hardware For_i loop (for slope-based HW timing).
    ablate: diagnostic-only comma list ('noh0','nospmm','nosbuild','noevict')
    that skips parts of the pipeline (output becomes garbage)."""
    assert zero_bias, "bf16 fast path requires zero biases"
    nchunk = NHALF * nch_per_half
    ne = nchunk * P
    moff, mw16 = _meta16_cols(nchunk)

    nc = bacc.Bacc(
        "TRN2", target_bir_lowering=False, debug=False, num_devices=NCORES
    )
    # per-core inputs
    xeT = nc.declare_dram_parameter("xeT", [P, ne], BF16, isOutput=False)
    meta16 = nc.declare_dram_parameter("meta16", [P, mw16], BF16, isOutput=False)
    meta32 = nc.declare_dram_parameter("meta32", [P, NHALF + 1 + nchunk], F32,
                                       isOutput=False)
    out = nc.declare_dram_parameter("out", [AGENTS_PER_CORE, EMB], F32,
                                    isOutput=True)

    with tile.TileContext(nc) as tc:
        with (
            tc.tile_pool(name="const", bufs=1) as cp,
            tc.tile_pool(name="io", bufs=2) as iop,
            tc.tile_pool(name="hs0p", bufs=5) as hs0p,
            tc.tile_pool(name="selp", bufs=8) as selp,
            tc.tile_pool(name="stage", bufs=2) as stp,
            tc.tile_pool(name="h0ps", bufs=3, space="PSUM") as h0psp,
            tc.tile_pool(name="aggtps", bufs=2, space="PSUM") as aggtpsp,
            tc.tile_pool(name="mlpps", bufs=1, space="PSUM") as mlppsp,
        ):
            def _body():
                emit_compute(
                    nc, cp, iop, hs0p, selp, stp, h0psp, aggtpsp, mlppsp,
                    nch_per_half, nchunk, ne, moff, mw16,
                    xeT, meta16, meta32, out, ablate,
                )

            if repeat == 1:
                _body()
            else:
                with tc.For_i(0, repeat, 1):
                    _body()
    nc.compile()
    return nc


def emit_compute(nc, cp, iop, hs0p, selp, stp, h0psp, aggtpsp, mlppsp,
                 nch_per_half, nchunk, ne, moff, mw16,
                 xeT, meta16, meta32, out, ablate=""):
    abl = set(a for a in ablate.split(",") if a)
    AG = AGENTS_PER_CORE
    if "empty" in abl:     # diagnostic: measure bare For_i loop overhead
        out_e = stp.tile([AG, EMB], F32, tag="outt")
        nc.vector.memset(out_e[:], 0.0)
        nc.sync.dma_start(out=out[:], in_=out_e[:])
        return
    KW = HID + HID + EMB               # 576: packed width per k-half of wblk

    # ---- all small constants ride ahead of the bulk xeT ----
    # double-buffered (iop pool) so iteration i+1's DMAs overlap iteration
    # i's compute in the For_i timing loop instead of serializing behind
    # its last reader
    meta16_t = iop.tile([P, mw16], BF16, tag="meta16")
    nc.sync.dma_start(out=meta16_t[:], in_=meta16[:])
    meta32_t = iop.tile([P, NHALF + 1 + nchunk], F32, tag="meta32")
    nc.sync.dma_start(out=meta32_t[:], in_=meta32[:])

    def mslice(name):
        lo, hi = moff[name]
        return meta16_t[:, lo:hi]

    dstl_t = meta32_t[:, NHALF + 1:]
    iota_t = mslice("iota")
    wblk_t = mslice("wblk").rearrange("p (c n) -> p c n", n=KW)
    wlin_t = mslice("wlin")
    a2t_t = mslice("a2t").rearrange("p (c n) -> p c n", n=AG)
    io1_t = meta32_t[:, 0:NHALF]       # per-slot in_norm*out_norm, per half
    in2_t = meta32_t[:AG, NHALF:NHALF + 1]   # per-agent in_norm

    # PE warm-up: keep the HAM activity window busy while DMAs stream in
    wu_t = cp.tile([P, P], BF16, tag="wu")
    nc.vector.memset(wu_t[:], 0.25)
    warm_ps = mlppsp.tile([P, HID], F32, tag="mlp", name="warm")
    for _w in range(3):
        nc.tensor.matmul(
            out=warm_ps[:, :P], lhsT=wu_t[:], rhs=wu_t[:],
            start=True, stop=True,
        )

    # ---- bulk x_e^T load, sliced so compute starts after slice 0 ----
    xeT_t = iop.tile([P, ne], BF16, tag="xeT")
    n_sl = 2
    sl = -(-nchunk // n_sl) * P   # slice width in columns, chunk-aligned
    for _s in range(n_sl):
        lo, hi = _s * sl, min((_s + 1) * sl, ne)
        if lo >= hi:
            break
        nc.sync.dma_start(out=xeT_t[:, lo:hi], in_=xeT[:, lo:hi])

    # ---- stage A: per half, accumulate aggT then h1 ----
    # h1 for half h-1 is emitted after half h's first pairs so the PE FIFO
    # never stalls on the DVE aggT evictions at half boundaries.
    hs1_t = stp.tile([P, NHALF, HID], BF16, tag="hs1")  # node-major
    assert nch_per_half % 2 == 0

    def emit_h1(aggT_prev, hprev):
        h1_ps = mlppsp.tile([P, HID], F32, tag="mlp", name=f"h1_{hprev}")
        aggT_sb = selp.tile([P, 2 * P], BF16, tag="daT")
        nc.vector.tensor_copy(out=aggT_sb[:], in_=aggT_prev[:])
        for k in range(HID // P):
            nc.tensor.matmul(
                out=h1_ps[:], lhsT=aggT_sb[:, k * P:(k + 1) * P],
                rhs=wblk_t[:, k, 0:HID],
                start=(k == 0), stop=(k == HID // P - 1),
            )
        nc.scalar.activation(
            hs1_t[:, hprev, :], h1_ps[:], AF.Relu, scale=io1_t[:, hprev:hprev + 1]
        )

    h0_pre = []
    if "noh0" in abl:     # 3 pre-written PSUM tiles the evictions can read
        for _i in range(3):
            t = h0psp.tile([P, 2 * HID], F32, tag="h0", name=f"h0pre{_i}")
            for u in range(2):
                nc.tensor.matmul(
                    out=t[:, u * HID:(u + 1) * HID],
                    lhsT=xeT_t[:, u * P:(u + 1) * P], rhs=wlin_t,
                    start=True, stop=True)
            h0_pre.append(t)

    prev = None
    for h in range(NHALF):
        # both f-halves of aggT side by side in one PSUM bank
        aggT_ps = aggtpsp.tile([P, 2 * P], F32, tag="aggT", name=f"aggT_{h}")
        if "nospmm" in abl:       # keep h1's producers defined
            for fh in range(2):
                nc.tensor.matmul(out=aggT_ps[:, fh * P:(fh + 1) * P],
                                 lhsT=iota_t, rhs=iota_t,
                                 start=True, stop=True)
        for jp in range(nch_per_half // 2):
            c0 = h * nch_per_half + 2 * jp
            if "noh0" in abl:
                h0_ps = h0_pre[jp % 3]
            else:
                h0_ps = h0psp.tile([P, 2 * HID], F32, tag="h0")
                for u in range(2):
                    nc.tensor.matmul(
                        out=h0_ps[:, u * HID:(u + 1) * HID],
                        lhsT=xeT_t[:, (c0 + u) * P:(c0 + u + 1) * P],
                        rhs=wlin_t,
                        start=True, stop=True,
                    )
            hs0_t = None
            if "noevict" not in abl:
                hs0_t = hs0p.tile([P, 2 * HID], BF16, tag="hs0")
                if jp % 2 == 1:
                    # every third pair's relu eviction runs on DVE to keep
                    # the ACT queue from becoming the bottleneck
                    nc.vector.tensor_scalar(
                        out=hs0_t[:], in0=h0_ps[:], scalar1=0.0, scalar2=None,
                        op0=ALU.max,
                    )
                else:
                    nc.scalar.activation(hs0_t[:], h0_ps[:], AF.Relu)
            if "nospmm" not in abl:
                for u in range(2):
                    c = c0 + u
                    j = 2 * jp + u
                    # S[e,d] = (iota[d] == dstl[e]): one DVE op, bf16 exact
                    if "nosbuild" in abl:
                        ss_t = iota_t
                    else:
                        ss_t = selp.tile([P, P], BF16, tag="ssel")
                        nc.vector.tensor_scalar(
                            out=ss_t[:], in0=iota_t,
                            scalar1=dstl_t[:, c:c + 1], scalar2=None,
                            op0=ALU.is_equal,
                        )
                    for fh in range(2):
                        lh = (hs0_t[:, u * HID + fh * P:u * HID + (fh + 1) * P]
                              if hs0_t is not None else iota_t[:])
                        # one accumulation group spans both column halves:
                        # start clears the whole bank; later MMs overwrite
                        # untouched elements via per-element has_written
                        nc.tensor.matmul(
                            out=aggT_ps[:, fh * P:(fh + 1) * P],
                            lhsT=lh,
                            rhs=ss_t[:] if ss_t is not iota_t else iota_t[:],
                            start=(j == 0 and fh == 0),
                            stop=(j == nch_per_half - 1 and fh == 1),
                        )
            if jp == 1 and prev is not None:
                emit_h1(*prev)
                prev = None
        prev = (aggT_ps, h)
    emit_h1(*prev)

    # ---- stage B: layer 2 on the 8 agent rows, feature-major ----
    out_ps = mlppsp.tile([AG, EMB], F32, tag="mlp", name="outps")
    h2rT_t = [None, None]
    for oh in range(2):
        a2T_ps = aggtpsp.tile([P, AG], F32, tag="aggT",
                              name=f"a2T{oh}")
        for h in range(NHALF):
            nc.tensor.matmul(
                out=a2T_ps[:],
                lhsT=hs1_t[:, h, oh * P:(oh + 1) * P],
                rhs=a2t_t[:, h, :],
                start=(h == 0), stop=(h == NHALF - 1),
            )
        a2T_sb = selp.tile([P, AG], BF16, tag="da2T", name=f"a2Tsb{oh}")
        nc.vector.tensor_copy(out=a2T_sb[:], in_=a2T_ps[:])
        h2rT_t[oh] = a2T_sb
    z2T_sb = [None, None]
    for oh in range(2):
        z2_ps = aggtpsp.tile([P, AG], F32, tag="aggT",
                             name=f"z2T{oh}")
        for kc in range(2):
            nc.tensor.matmul(
                out=z2_ps[:],
                lhsT=wblk_t[:, kc, HID + oh * P:HID + (oh + 1) * P],
                rhs=h2rT_t[kc][:],
                start=(kc == 0), stop=(kc == 1),
            )
        zr_t = hs0p.tile([P, AG], BF16, tag="hb", name=f"z2r{oh}")
        nc.scalar.activation(zr_t[:], z2_ps[:], AF.Relu)
        z2T_sb[oh] = zr_t
    for oh in range(2):
        nc.tensor.matmul(
            out=out_ps[:], lhsT=z2T_sb[oh][:],
            rhs=wblk_t[:, oh, 2 * HID:2 * HID + EMB],
            start=(oh == 0), stop=(oh == 1),
        )
    out_t = stp.tile([AG, EMB], F32, tag="outt")
    nc.scalar.activation(out_t[:], out_ps[:], AF.Copy, scale=in2_t[:, 0:1])
    nc.sync.dma_start(out=out[:], in_=out_t[:])


def prepare_inputs(x, src, dst):
    """Host-side integer index preprocessing + sharding. Agents are
    LPT-assigned to cores (8 each) to balance cone edge counts, and S1
    nodes are LPT-assigned to the 3 dst halves to balance chunk counts."""
    deg_out = np.bincount(src, minlength=TOTAL).astype(np.float32)
    deg_in = np.bincount(dst, minlength=TOTAL).astype(np.float32)

    g = dst // NPG                     # graph id of each edge's dst
    is_agent = (dst % NPG) == 0
    g2 = g[is_agent]
    s2_all = src[is_agent]

    # per-agent cone load = sum of in-degrees over its distinct sources
    loads = np.zeros(B, np.int64)
    for a in range(B):
        loads[a] = deg_in[np.unique(s2_all[g2 == a])].sum()
    bins = [[] for _ in range(NCORES)]
    bl = np.zeros(NCORES, np.int64)
    for a in np.argsort(-loads):
        cands = [i for i in range(NCORES) if len(bins[i]) < AGENTS_PER_CORE]
        i = min(cands, key=lambda i: bl[i])
        bins[i].append(int(a))
        bl[i] += loads[a]

    cores = []
    agent_rows = []                     # global output row per concat position
    nch_needed = 1
    for c in range(NCORES):
        agents_g = bins[c]              # graph ids owned by this core
        agent_rows.extend(agents_g)
        # --- layer-2 edge bucket: dst is an agent owned by this core ---
        am = np.zeros(B, bool)
        am[agents_g] = True
        m2 = is_agent & am[g]
        e2_src = src[m2]
        gl = np.full(B, -1, np.int64)
        gl[agents_g] = np.arange(AGENTS_PER_CORE)
        e2_ag = gl[g[m2]]
        s1 = np.unique(e2_src)
        m1c = s1.size
        assert m1c <= NHALF * 127, f"S1 overflow: {m1c}"
        # slot: LPT nodes into halves by in-degree (127 usable slots each,
        # slot 127 of each half is the pad/trash slot)
        hload = np.zeros(NHALF, np.int64)
        hfill = np.zeros(NHALF, np.int64)
        slot = np.empty(m1c, np.int64)
        d1 = deg_in[s1].astype(np.int64)
        for i in np.argsort(-d1):
            cands = [hh for hh in range(NHALF) if hfill[hh] < P - 1]
            hh = min(cands, key=lambda hh: hload[hh])
            slot[i] = hh * P + hfill[hh]
            hfill[hh] += 1
            hload[hh] += d1[i]
        # lookup: global node id -> slot
        loc = np.full(TOTAL, -1, dtype=np.int64)
        loc[s1] = slot
        a2t = np.zeros((M1, AGENTS_PER_CORE), dtype=np.float32)
        np.add.at(a2t, (loc[e2_src], e2_ag), 1.0)

        indeg1 = np.zeros(M1, np.float32)
        outdeg1 = np.zeros(M1, np.float32)
        indeg1[loc[s1]] = deg_in[s1]
        outdeg1[loc[s1]] = deg_out[s1]
        agents = np.asarray(agents_g, np.int64) * NPG
        indeg2 = deg_in[agents].reshape(AGENTS_PER_CORE, 1)

        # --- layer-1 edge bucket: dst in S1 ---
        dl = loc[dst]
        sel = dl >= 0
        e1_src = src[sel]
        e1_slot = dl[sel]
        halves = []
        for h in range(NHALF):
            hm = (e1_slot // P) == h
            halves.append((e1_src[hm], e1_slot[hm] - h * P))
            nch_needed = max(nch_needed, -(-halves[h][0].size // P))
        cores.append(dict(a2t=a2t, indeg1=indeg1, outdeg1=outdeg1,
                          indeg2=indeg2, halves=halves))
    return cores, deg_out, nch_needed, np.asarray(agent_rows, np.int64)


def pack_core(core, x, out_norm, nch_per_half, w_lin, wblk, np16):
    nchunk = NHALF * nch_per_half
    ne = nchunk * P
    moff, mw16 = _meta16_cols(nchunk)
    xe = np.zeros((ne, IN_DIM), dtype=np.float32)
    dstl_e = np.full(ne, P - 1, dtype=np.float32)  # pads -> trash slot 127
    for h, (hsrc, hslot) in enumerate(core["halves"]):
        base = h * nch_per_half * P
        k = hsrc.size
        # out-degree norm folded into x (exact: relu(s*z) = s*relu(z), s>0)
        xe[base:base + k] = x[hsrc] * out_norm[hsrc][:, None]
        dstl_e[base:base + k] = hslot
    meta16 = np.zeros((P, mw16), dtype=np.float32)

    def put(name, arr):
        lo, hi = moff[name]
        meta16[:arr.shape[0], lo:hi] = arr

    put("iota", np.broadcast_to(np.arange(P, dtype=np.float32), (P, P)))
    put("a2t", core["a2t"].reshape(NHALF, P, AGENTS_PER_CORE)
        .transpose(1, 0, 2).reshape(P, NHALF * AGENTS_PER_CORE))
    put("wlin", np.asarray(w_lin, np.float32))
    put("wblk", wblk)

    # meta32: io1 (per-slot in*out norm, [128, 3]) + in2 (per-agent, rows 0-7)
    i1 = 1.0 / np.sqrt(np.maximum(core["indeg1"], 1.0))
    o1 = 1.0 / np.sqrt(np.maximum(core["outdeg1"], 1.0))
    io1 = (i1 * o1).reshape(NHALF, P).T
    meta32 = np.zeros((P, NHALF + 1 + nchunk), dtype=np.float32)
    meta32[:, 0:NHALF] = io1
    meta32[:AGENTS_PER_CORE, NHALF:NHALF + 1] = \
        1.0 / np.sqrt(np.maximum(core["indeg2"], 1.0))
    meta32[:, NHALF + 1:] = dstl_e.reshape(nchunk, P).T
    return dict(xeT=np.ascontiguousarray(xe.T).astype(np16),
                meta16=meta16.astype(np16), meta32=meta32)


def pack_weights(w_c0, w_c1, w_emb):
    """[128, 2, wc0(256)|wc1(256)|wemb(64)] k-major, fp32, flattened."""
    KW = HID + HID + EMB
    wb = np.zeros((P, 2, KW), np.float32)
    for k in range(2):
        wb[:, k, 0:HID] = np.asarray(w_c0, np.float32)[k * P:(k + 1) * P]
        wb[:, k, HID:2 * HID] = np.asarray(w_c1, np.float32)[k * P:(k + 1) * P]
        wb[:, k, 2 * HID:] = np.asarray(w_emb, np.float32)[k * P:(k + 1) * P]
    return wb.reshape(P, 2 * KW)


def assemble_out(core_outs, agent_rows):
    """Scatter per-core [8, EMB] outputs back to global agent row order."""
    full = np.empty((B, EMB), np.float32)
    full[agent_rows] = np.concatenate(core_outs, axis=0)
    return full


def make_in_maps(x, src, dst, w_lin, b_lin, w_c0, b_c0, w_c1, b_c1,
                 w_emb, b_emb):
    """Host preprocessing -> (in_maps, nch_per_half, zero_bias, agent_rows)."""
    np16 = mybir.dt.np(BF16)
    x = np.asarray(x, dtype=np.float32)
    src = np.asarray(src).astype(np.int64)
    dst = np.asarray(dst).astype(np.int64)
    cores, deg_out, nch_per_half, agent_rows = prepare_inputs(x, src, dst)
    nch_per_half += nch_per_half % 2   # paired-chunk path needs even count
    out_norm = 1.0 / np.sqrt(np.maximum(deg_out, 1.0))
    wblk = pack_weights(w_c0, w_c1, w_emb)
    in_maps = []
    for c in range(NCORES):
        m = pack_core(cores[c], x, out_norm, nch_per_half, w_lin, wblk, np16)
        in_maps.append(m)
    return in_maps, nch_per_half, True, agent_rows


def _host_fallback(x, src, dst, w_lin, b_lin, w_c0, b_c0, w_c1, b_c1,
                   w_emb, b_emb):
    """Numpy fallback for nonzero biases (never hit for this problem)."""
    x = np.asarray(x, np.float32)
    src = np.asarray(src).astype(np.int64)
    dst = np.asarray(dst).astype(np.int64)
    deg_out = np.bincount(src, minlength=TOTAL).astype(np.float32)
    deg_in = np.bincount(dst, minlength=TOTAL).astype(np.float32)
    on = np.maximum(deg_out, 1.0) ** -0.5
    inn = np.maximum(deg_in, 1.0) ** -0.5

    def conv(h, W, bb):
        hs = h * on[:, None]
        agg = np.zeros_like(h)
        for f in range(h.shape[1]):
            agg[:, f] = np.bincount(dst, weights=hs[src, f].astype(np.float64),
                                    minlength=TOTAL)
        return (agg * inn[:, None]) @ W + bb

    h = np.maximum(x @ w_lin + b_lin, 0.0)
    h = np.maximum(conv(h, w_c0, b_c0), 0.0)
    h = np.maximum(conv(h, w_c1, b_c1), 0.0)
    return (h[::NPG] @ w_emb + b_emb).astype(np.float32)


def kernel(x, src, dst, num_nodes, nodes_per_graph,
           w_lin, b_lin, w_c0, b_c0, w_c1, b_c1, w_emb, b_emb,
           _debug=None) -> np.ndarray:
    from concourse.bass_utils import run_bass_kernel_spmd

    assert int(num_nodes) == TOTAL and int(nodes_per_graph) == NPG
    if (np.any(np.asarray(b_lin)) or np.any(np.asarray(b_c0))
            or np.any(np.asarray(b_c1)) or np.any(np.asarray(b_emb))):
        return _host_fallback(x, src, dst, w_lin, b_lin, w_c0, b_c0,
                              w_c1, b_c1, w_emb, b_emb)
    in_maps, nch_per_half, zero_bias, agent_rows = make_in_maps(
        x, src, dst, w_lin, b_lin, w_c0, b_c0, w_c1, b_c1, w_emb, b_emb)

    nc = build_program(nch_per_half, zero_bias=zero_bias)
    core_ids = list(range(NCORES))
    if _debug is not None:
        _debug["nc"] = nc
        _debug["in_maps"] = in_maps
        _debug["nch_per_half"] = nch_per_half
    res = run_bass_kernel_spmd(nc, in_maps, core_ids)
    return assemble_out([res.results[c]["out"] for c in range(NCORES)],
                        agent_rows)


# revision 51
# speedup vs baseline: 2.0793x; 1.0284x over previous
"""Trainium2 Bass kernel for nn_AggregateGCN (3-layer GCN, batched graph,
agent-node readout).

Math (reference): deg-normalized GraphConv x2 on top of a linear+relu input
projection, then a final projection of the 64 agent rows (nodes 0, N, 2N, ...).
Only the 64 agent rows of the last conv are read, so the exact computation
is the backward dependency cone:
  layer2 needs edges into the 64 agents (~2k edges -> ~2k distinct sources S1)
  layer1 needs edges into S1 (~64k edges), with per-edge h0 = relu(x@w_lin+b)

Sharding: agents are LPT-assigned to cores (8 each, balancing cone edge
counts) with each core's full cone replicated -> zero cross-device traffic;
the host scatters the per-core [8, 64] outputs back to global row order.

All biases are zero for this problem, which makes the whole pipeline
positively homogeneous: relu(s*x @ W) = s*relu(x @ W) for s>0. The host
therefore folds the per-edge out-degree norm into x itself and ships
  xeT = (x[src] * out_norm[src])^T  in bf16
so the device does NO degree math at all. All matmuls run in bf16
(fp32 matmul is 4 cycles/row on TRN2; bf16 is 1), accumulating in fp32 PSUM.

On device per core (per 128-edge chunk):
  - h0 = xe_chunk.T @ wlin           (PE, bf16, N=256)
  - hs0 = relu(h0)                   (ACT or DVE, alternating; bf16 out)
  - S[e,d] = (iota[d]==dstl[e])      (DVE tensor_scalar, bf16, exact 0/1)
  - aggT[f,d] += hs0_fhalf.T @ S     (PE, accumulated in PSUM per half)
then per half: h1 = aggT.T @ wc0, hs1 = relu(io1 * h1) with io1 =
in_norm*out_norm per slot (host-computed; legal because relu(c*z)=c*relu(z)
for c>0), and a small feature-major stage B for the 8 agent rows.
"""
import sys

sys.path.insert(0, "/opt/trn_rl_repo")

import numpy as np
import concourse.bass as bass
import concourse.bacc as bacc
import concourse.mybir as mybir
import concourse.tile as tile

F32 = mybir.dt.float32
BF16 = mybir.dt.bfloat16
AF = mybir.ActivationFunctionType
ALU = mybir.AluOpType

# problem constants (fixed by the spec)
B = 64          # graphs
NPG = 2048      # nodes per graph
TOTAL = B * NPG
IN_DIM = 128
HID = 256
EMB = 64
NCORES = 8
AGENTS_PER_CORE = B // NCORES      # 8
M1 = 384                           # padded S1 slots per core (3 halves of 128)
NHALF = M1 // 128                  # 3
P = 128


def _meta16_cols(nchunk):
    """Column offsets inside the packed bf16 meta tensor [128, MW]."""
    off = {}
    o = 0
    for name, w in [("iota", P),
                    ("a2t", NHALF * AGENTS_PER_CORE), ("wlin", HID),
                    ("wblk", 2 * (HID + HID + EMB))]:
        off[name] = (o, o + w)
        o += w
    return off, o


def build_program(nch_per_half: int, repeat: int = 1,
                  zero_bias: bool = True, ablate: str = "") -> bass.Bass:
    """One SPMD program; per-core data differs via in_maps. repeat>1 re-runs
    the whole compute in a hardware For_i loop (for slope-based HW timing).
    ablate: diagnostic-only comma list ('noh0','nospmm','nosbuild','noevict')
    that skips parts of the pipeline (output becomes garbage)."""
    assert zero_bias, "bf16 fast path requires zero biases"
    nchunk = NHALF * nch_per_half
    ne = nchunk * P
    moff, mw16 = _meta16_cols(nchunk)

    nc = bacc.Bacc(
        "TRN2", target_bir_lowering=False, debug=False, num_devices=NCORES
    )
    # per-core inputs
    xeT = nc.declare_dram_parameter("xeT", [P, ne], BF16, isOutput=False)
    meta16 = nc.declare_dram_parameter("meta16", [P, mw16], BF16, isOutput=False)
    meta32 = nc.declare_dram_parameter("meta32", [P, NHALF + 1 + nchunk], F32,
                                       isOutput=False)
    out = nc.declare_dram_parameter("out", [AGENTS_PER_CORE, EMB], F32,
                                    isOutput=True)

    with tile.TileContext(nc) as tc:
        with (
            tc.tile_pool(name="const", bufs=1) as cp,
            tc.tile_pool(name="io", bufs=2) as iop,
            tc.tile_pool(name="hs0p", bufs=5) as hs0p,
            tc.tile_pool(name="selp", bufs=8) as selp,
            tc.tile_pool(name="stage", bufs=2) as stp,
            tc.tile_pool(name="h0ps", bufs=3, space="PSUM") as h0psp,
            tc.tile_pool(name="aggtps", bufs=2, space="PSUM") as aggtpsp,
            tc.tile_pool(name="mlpps", bufs=1, space="PSUM") as mlppsp,
        ):
            def _body():
                emit_compute(
                    nc, cp, iop, hs0p, selp, stp, h0psp, aggtpsp, mlppsp,
                    nch_per_half, nchunk, ne, moff, mw16,
                    xeT, meta16, meta32, out, ablate,
                )

            if repeat == 1:
                _body()
            else:
                with tc.For_i(0, repeat, 1):
                    _body()
    nc.compile()
    return nc


def emit_compute(nc, cp, iop, hs0p, selp, stp, h0psp, aggtpsp, mlppsp,
                 nch_per_half, nchunk, ne, moff, mw16,
                 xeT, meta16, meta32, out, ablate=""):
    abl = set(a for a in ablate.split(",") if a)
    AG = AGENTS_PER_CORE
    if "empty" in abl:     # diagnostic: measure bare For_i loop overhead
        out_e = stp.tile([AG, EMB], F32, tag="outt")
        nc.vector.memset(out_e[:], 0.0)
        nc.sync.dma_start(out=out[:], in_=out_e[:])
        return
    KW = HID + HID + EMB               # 576: packed width per k-half of wblk

    # ---- all small constants ride ahead of the bulk xeT ----
    # double-buffered (iop pool) so iteration i+1's DMAs overlap iteration
    # i's compute in the For_i timing loop instead of serializing behind
    # its last reader
    meta16_t = iop.tile([P, mw16], BF16, tag="meta16")
    nc.sync.dma_start(out=meta16_t[:], in_=meta16[:])
    meta32_t = iop.tile([P, NHALF + 1 + nchunk], F32, tag="meta32")
    nc.sync.dma_start(out=meta32_t[:], in_=meta32[:])

    def mslice(name):
        lo, hi = moff[name]
        return meta16_t[:, lo:hi]

    dstl_t = meta32_t[:, NHALF + 1:]
    iota_t = mslice("iota")
    wblk_t = mslice("wblk").rearrange("p (c n) -> p c n", n=KW)
    wlin_t = mslice("wlin")
    a2t_t = mslice("a2t").rearrange("p (c n) -> p c n", n=AG)
    io1_t = meta32_t[:, 0:NHALF]       # per-slot in_norm*out_norm, per half
    in2_t = meta32_t[:AG, NHALF:NHALF + 1]   # per-agent in_norm

    # PE warm-up: keep the HAM activity window busy while DMAs stream in
    wu_t = cp.tile([P, P], BF16, tag="wu")
    nc.vector.memset(wu_t[:], 0.25)
    warm_ps = mlppsp.tile([P, HID], F32, tag="mlp", name="warm")
    for _w in range(3):
        nc.tensor.matmul(
            out=warm_ps[:, :P], lhsT=wu_t[:], rhs=wu_t[:],
            start=True, stop=True,
        )

    # ---- bulk x_e^T load, sliced so compute starts after slice 0 ----
    xeT_t = iop.tile([P, ne], BF16, tag="xeT")
    n_sl = 2
    sl = -(-nchunk // n_sl) * P   # slice width in columns, chunk-aligned
    for _s in range(n_sl):
        lo, hi = _s * sl, min((_s + 1) * sl, ne)
        if lo >= hi:
            break
        nc.sync.dma_start(out=xeT_t[:, lo:hi], in_=xeT[:, lo:hi])

    # ---- stage A: per half, accumulate aggT then h1 ----
    # h1 for half h-1 is emitted after half h's first pairs so the PE FIFO
    # never stalls on the DVE aggT evictions at half boundaries.
    hs1_t = stp.tile([P, NHALF, HID], BF16, tag="hs1")  # node-major
    assert nch_per_half % 2 == 0

    def emit_h1(aggT_prev, hprev):
        h1_ps = mlppsp.tile([P, HID], F32, tag="mlp", name=f"h1_{hprev}")
        aggT_sb = selp.tile([P, 2 * P], BF16, tag="daT")
        nc.vector.tensor_copy(out=aggT_sb[:], in_=aggT_prev[:])
        for k in range(HID // P):
            nc.tensor.matmul(
                out=h1_ps[:], lhsT=aggT_sb[:, k * P:(k + 1) * P],
                rhs=wblk_t[:, k, 0:HID],
                start=(k == 0), stop=(k == HID // P - 1),
            )
        nc.scalar.activation(
            hs1_t[:, hprev, :], h1_ps[:], AF.Relu, scale=io1_t[:, hprev:hprev + 1]
        )

    h0_pre = []
    if "noh0" in abl:     # 3 pre-written PSUM tiles the evictions can read
        for _i in range(3):
            t = h0psp.tile([P, 2 * HID], F32, tag="h0", name=f"h0pre{_i}")
            for u in range(2):
                nc.tensor.matmul(
                    out=t[:, u * HID:(u + 1) * HID],
                    lhsT=xeT_t[:, u * P:(u + 1) * P], rhs=wlin_t,
                    start=True, stop=True)
            h0_pre.append(t)

    prev = None
    for h in range(NHALF):
        # both f-halves of aggT side by side in one PSUM bank
        aggT_ps = aggtpsp.tile([P, 2 * P], F32, tag="aggT", name=f"aggT_{h}")
        if "nospmm" in abl:       # keep h1's producers defined
            for fh in range(2):
                nc.tensor.matmul(out=aggT_ps[:, fh * P:(fh + 1) * P],
                                 lhsT=iota_t, rhs=iota_t,
                                 start=True, stop=True)
        for jp in range(nch_per_half // 2):
            c0 = h * nch_per_half + 2 * jp
            if "noh0" in abl:
                h0_ps = h0_pre[jp % 3]
            else:
                h0_ps = h0psp.tile([P, 2 * HID], F32, tag="h0")
                for u in range(2):
                    nc.tensor.matmul(
                        out=h0_ps[:, u * HID:(u + 1) * HID],
                        lhsT=xeT_t[:, (c0 + u) * P:(c0 + u + 1) * P],
                        rhs=wlin_t,
                        start=True, stop=True,
                    )
            hs0_t = None
            if "noevict" not in abl:
                hs0_t = hs0p.tile([P, 2 * HID], BF16, tag="hs0")
                if jp % 3 == 2:
                    # every third pair's relu eviction runs on DVE to keep
                    # the ACT queue from becoming the bottleneck
                    nc.vector.tensor_scalar(
                        out=hs0_t[:], in0=h0_ps[:], scalar1=0.0, scalar2=None,
                        op0=ALU.max,
                    )
                else:
                    nc.scalar.activation(hs0_t[:], h0_ps[:], AF.Relu)
            if "nospmm" not in abl:
                for u in range(2):
                    c = c0 + u
                    j = 2 * jp + u
                    # S[e,d] = (iota[d] == dstl[e]): one DVE op, bf16 exact
                    if "nosbuild" in abl:
                        ss_t = iota_t
                    else:
                        ss_t = selp.tile([P, P], BF16, tag="ssel")
                        nc.vector.tensor_scalar(
                            out=ss_t[:], in0=iota_t,
                            scalar1=dstl_t[:, c:c + 1], scalar2=None,
                            op0=ALU.is_equal,
                        )
                    for fh in range(2):
                        lh = (hs0_t[:, u * HID + fh * P:u * HID + (fh + 1) * P]
                              if hs0_t is not None else iota_t[:])
                        # one accumulation group spans both column halves:
                        # start clears the whole bank; later MMs overwrite
                        # untouched elements via per-element has_written
                        nc.tensor.matmul(
                            out=aggT_ps[:, fh * P:(fh + 1) * P],
                            lhsT=lh,
                            rhs=ss_t[:] if ss_t is not iota_t else iota_t[:],
                            start=(j == 0 and fh == 0),
                            stop=(j == nch_per_half - 1 and fh == 1),
                        )
            if jp == 1 and prev is not None:
                emit_h1(*prev)
                prev = None
        prev = (aggT_ps, h)
    emit_h1(*prev)

    # ---- stage B: layer 2 on the 8 agent rows, feature-major ----
    out_ps = mlppsp.tile([AG, EMB], F32, tag="mlp", name="outps")
    h2rT_t = [None, None]
    for oh in range(2):
        a2T_ps = aggtpsp.tile([P, AG], F32, tag="aggT",
                              name=f"a2T{oh}")
        for h in range(NHALF):
            nc.tensor.matmul(
                out=a2T_ps[:],
                lhsT=hs1_t[:, h, oh * P:(oh + 1) * P],
                rhs=a2t_t[:, h, :],
                start=(h == 0), stop=(h == NHALF - 1),
            )
        a2T_sb = selp.tile([P, AG], BF16, tag="da2T", name=f"a2Tsb{oh}")
        nc.vector.tensor_copy(out=a2T_sb[:], in_=a2T_ps[:])
        h2rT_t[oh] = a2T_sb
    z2T_sb = [None, None]
    for oh in range(2):
        z2_ps = aggtpsp.tile([P, AG], F32, tag="aggT",
                             name=f"z2T{oh}")
        for kc in range(2):
            nc.tensor.matmul(
                out=z2_ps[:],
                lhsT=wblk_t[:, kc, HID + oh * P:HID + (oh + 1) * P],
                rhs=h2rT_t[kc][:],
                start=(kc == 0), stop=(kc == 1),
            )
        zr_t = hs0p.tile([P, AG], BF16, tag="hb", name=f"z2r{oh}")
        nc.scalar.activation(zr_t[:], z2_ps[:], AF.Relu)
        z2T_sb[oh] = zr_t
    for oh in range(2):
        nc.tensor.matmul(
            out=out_ps[:], lhsT=z2T_sb[oh][:],
            rhs=wblk_t[:, oh, 2 * HID:2 * HID + EMB],
            start=(oh == 0), stop=(oh == 1),
        )
    out_t = stp.tile([AG, EMB], F32, tag="outt")
    nc.scalar.activation(out_t[:], out_ps[:], AF.Copy, scale=in2_t[:, 0:1])
    nc.sync.dma_start(out=out[:], in_=out_t[:])


def prepare_inputs(x, src, dst):
    """Host-side integer index preprocessing + sharding. Agents are
    LPT-assigned to cores (8 each) to balance cone edge counts, and S1
    nodes are LPT-assigned to the 3 dst halves to balance chunk counts."""
    deg_out = np.bincount(src, minlength=TOTAL).astype(np.float32)
    deg_in = np.bincount(dst, minlength=TOTAL).astype(np.float32)

    g = dst // NPG                     # graph id of each edge's dst
    is_agent = (dst % NPG) == 0
    g2 = g[is_agent]
    s2_all = src[is_agent]

    # per-agent cone load = sum of in-degrees over its distinct sources
    loads = np.zeros(B, np.int64)
    for a in range(B):
        loads[a] = deg_in[np.unique(s2_all[g2 == a])].sum()
    bins = [[] for _ in range(NCORES)]
    bl = np.zeros(NCORES, np.int64)
    for a in np.argsort(-loads):
        cands = [i for i in range(NCORES) if len(bins[i]) < AGENTS_PER_CORE]
        i = min(cands, key=lambda i: bl[i])
        bins[i].append(int(a))
        bl[i] += loads[a]

    cores = []
    agent_rows = []                     # global output row per concat position
    nch_needed = 1
    for c in range(NCORES):
        agents_g = bins[c]              # graph ids owned by this core
        agent_rows.extend(agents_g)
        # --- layer-2 edge bucket: dst is an agent owned by this core ---
        am = np.zeros(B, bool)
        am[agents_g] = True
        m2 = is_agent & am[g]
        e2_src = src[m2]
        gl = np.full(B, -1, np.int64)
        gl[agents_g] = np.arange(AGENTS_PER_CORE)
        e2_ag = gl[g[m2]]
        s1 = np.unique(e2_src)
        m1c = s1.size
        assert m1c <= NHALF * 127, f"S1 overflow: {m1c}"
        # slot: LPT nodes into halves by in-degree (127 usable slots each,
        # slot 127 of each half is the pad/trash slot)
        hload = np.zeros(NHALF, np.int64)
        hfill = np.zeros(NHALF, np.int64)
        slot = np.empty(m1c, np.int64)
        d1 = deg_in[s1].astype(np.int64)
        for i in np.argsort(-d1):
            cands = [hh for hh in range(NHALF) if hfill[hh] < P - 1]
            hh = min(cands, key=lambda hh: hload[hh])
            slot[i] = hh * P + hfill[hh]
            hfill[hh] += 1
            hload[hh] += d1[i]
        # lookup: global node id -> slot
        loc = np.full(TOTAL, -1, dtype=np.int64)
        loc[s1] = slot
        a2t = np.zeros((M1, AGENTS_PER_CORE), dtype=np.float32)
        np.add.at(a2t, (loc[e2_src], e2_ag), 1.0)

        indeg1 = np.zeros(M1, np.float32)
        outdeg1 = np.zeros(M1, np.float32)
        indeg1[loc[s1]] = deg_in[s1]
        outdeg1[loc[s1]] = deg_out[s1]
        agents = np.asarray(agents_g, np.int64) * NPG
        indeg2 = deg_in[agents].reshape(AGENTS_PER_CORE, 1)

        # --- layer-1 edge bucket: dst in S1 ---
        dl = loc[dst]
        sel = dl >= 0
        e1_src = src[sel]
        e1_slot = dl[sel]
        halves = []
        for h in range(NHALF):
            hm = (e1_slot // P) == h
            halves.append((e1_src[hm], e1_slot[hm] - h * P))
            nch_needed = max(nch_needed, -(-halves[h][0].size // P))
        cores.append(dict(a2t=a2t, indeg1=indeg1, outdeg1=outdeg1,
                          indeg2=indeg2, halves=halves))
    return cores, deg_out, nch_needed, np.asarray(agent_rows, np.int64)


def pack_core(core, x, out_norm, nch_per_half, w_lin, wblk, np16):
    nchunk = NHALF * nch_per_half
    ne = nchunk * P
    moff, mw16 = _meta16_cols(nchunk)
    xe = np.zeros((ne, IN_DIM), dtype=np.float32)
    dstl_e = np.full(ne, P - 1, dtype=np.float32)  # pads -> trash slot 127
    for h, (hsrc, hslot) in enumerate(core["halves"]):
        base = h * nch_per_half * P
        k = hsrc.size
        # out-degree norm folded into x (exact: relu(s*z) = s*relu(z), s>0)
        xe[base:base + k] = x[hsrc] * out_norm[hsrc][:, None]
        dstl_e[base:base + k] = hslot
    meta16 = np.zeros((P, mw16), dtype=np.float32)

    def put(name, arr):
        lo, hi = moff[name]
        meta16[:arr.shape[0], lo:hi] = arr

    put("iota", np.broadcast_to(np.arange(P, dtype=np.float32), (P, P)))
    put("a2t", core["a2t"].reshape(NHALF, P, AGENTS_PER_CORE)
        .transpose(1, 0, 2).reshape(P, NHALF * AGENTS_PER_CORE))
    put("wlin", np.asarray(w_lin, np.float32))
    put("wblk", wblk)

    # meta32: io1 (per-slot in*out norm, [128, 3]) + in2 (per-agent, rows 0-7)
    i1 = 1.0 / np.sqrt(np.maximum(core["indeg1"], 1.0))
    o1 = 1.0 / np.sqrt(np.maximum(core["outdeg1"], 1.0))
    io1 = (i1 * o1).reshape(NHALF, P).T
    meta32 = np.zeros((P, NHALF + 1 + nchunk), dtype=np.float32)
    meta32[:, 0:NHALF] = io1
    meta32[:AGENTS_PER_CORE, NHALF:NHALF + 1] = \
        1.0 / np.sqrt(np.maximum(core["indeg2"], 1.0))
    meta32[:, NHALF + 1:] = dstl_e.reshape(nchunk, P).T
    return dict(xeT=np.ascontiguousarray(xe.T).astype(np16),
                meta16=meta16.astype(np16), meta32=meta32)


def pack_weights(w_c0, w_c1, w_emb):
    """[128, 2, wc0(256)|wc1(256)|wemb(64)] k-major, fp32, flattened."""
    KW = HID + HID + EMB
    wb = np.zeros((P, 2, KW), np.float32)
    for k in range(2):
        wb[:, k, 0:HID] = np.asarray(w_c0, np.float32)[k * P:(k + 1) * P]
        wb[:, k, HID:2 * HID] = np.asarray(w_c1, np.float32)[k * P:(k + 1) * P]
        wb[:, k, 2 * HID:] = np.asarray(w_emb, np.float32)[k * P:(k + 1) * P]
    return wb.reshape(P, 2 * KW)


def assemble_out(core_outs, agent_rows):
    """Scatter per-core [8, EMB] outputs back to global agent row order."""
    full = np.empty((B, EMB), np.float32)
    full[agent_rows] = np.concatenate(core_outs, axis=0)
    return full


def make_in_maps(x, src, dst, w_lin, b_lin, w_c0, b_c0, w_c1, b_c1,
                 w_emb, b_emb):
    """Host preprocessing -> (in_maps, nch_per_half, zero_bias, agent_rows)."""
    np16 = mybir.dt.np(BF16)
    x = np.asarray(x, dtype=np.float32)
    src = np.asarray(src).astype(np.int64)
    dst = np.asarray(dst).astype(np.int64)
    cores, deg_out, nch_per_half, agent_rows = prepare_inputs(x, src, dst)
    nch_per_half += nch_per_half % 2   # paired-chunk path needs even count
    out_norm = 1.0 / np.sqrt(np.maximum(deg_out, 1.0))
    wblk = pack_weights(w_c0, w_c1, w_emb)
    in_maps = []
    for c in range(NCORES):
        m = pack_core(cores[c], x, out_norm, nch_per_half, w_lin, wblk, np16)
        in_maps.append(m)
    return in_maps, nch_per_half, True, agent_rows


def _host_fallback(x, src, dst, w_lin, b_lin, w_c0, b_c0, w_c1, b_c1,
                   w_emb, b_emb):
    """Numpy fallback for nonzero biases (never hit for this problem)."""
    x = np.asarray(x, np.float32)
    src = np.asarray(src).astype(np.int64)
    dst = np.asarray(dst).astype(np.int64)
    deg_out = np.bincount(src, minlength=TOTAL).astype(np.float32)
    deg_in = np.bincount(dst, minlength=TOTAL).astype(np.float32)
    on = np.maximum(deg_out, 1.0) ** -0.5
    inn = np.maximum(deg_in, 1.0) ** -0.5

    def conv(h, W, bb):
        hs = h * on[:, None]
        agg = np.zeros_like(h)
        for f in range(h.shape[1]):
            agg[:, f] = np.bincount(dst, weights=hs[src, f].astype(np.float64),
                                    minlength=TOTAL)
        return (agg * inn[:, None]) @ W + bb

    h = np.maximum(x @ w_lin + b_lin, 0.0)
    h = np.maximum(conv(h, w_c0, b_c0), 0.0)
    h = np.maximum(conv(h, w_c1, b_c1), 0.0)
    return (h[::NPG] @ w_emb + b_emb).astype(np.float32)


def kernel(x, src, dst, num_nodes, nodes_per_graph,
           w_lin, b_lin, w_c0, b_c0, w_c1, b_c1, w_emb, b_emb,
           _debug=None) -> np.ndarray:
    from concourse.bass_utils import run_bass_kernel_spmd

    assert int(num_nodes) == TOTAL and int(nodes_per_graph) == NPG
    if (np.any(np.asarray(b_lin)) or np.any(np.asarray(b_c0))
            or np.any(np.asarray(b_c1)) or np.any(np.asarray(b_emb))):
        return _host_fallback(x, src, dst, w_lin, b_lin, w_c0, b_c0,
                              w_c1, b_c1, w_emb, b_emb)
    in_maps, nch_per_half, zero_bias, agent_rows = make_in_maps(
        x, src, dst, w_lin, b_lin, w_c0, b_c0, w_c1, b_c1, w_emb, b_emb)

    nc = build_program(nch_per_half, zero_bias=zero_bias)
    core_ids = list(range(NCORES))
    if _debug is not None:
        _debug["nc"] = nc
        _debug["in_maps"] = in_maps
        _debug["nch_per_half"] = nch_per_half
    res = run_bass_kernel_spmd(nc, in_maps, core_ids)
    return assemble_out([res.results[c]["out"] for c in range(NCORES)],
                        agent_rows)
